# revision 9
# baseline (speedup 1.0000x reference)
"""Trainium2 Bass kernel for a 3-layer binarized MLP (BNN) with BatchNorm.

Math (reference):
  layer(x, W, a):  y = x_bin @ sign(W).T ; bn = (y - mean)/sqrt(var + eps) over
  the GLOBAL batch; p = prelu(bn, a); out = sign(p) (except last layer).

Key identities used:
  * sign(prelu((y - mu)/std)) == sign(y - mu)   (std > 0, a > 0) -> layers 1,2
    need only the global per-feature mean, not the variance.
  * mean(y) = mean(x_in) @ sign(W).T -> the cross-core all-reduce of the input
    sums can be computed before/while the layer's matmuls run.
  * layers 2,3 inputs are exactly {-1,+1} so bf16 matmuls are exact.
  * layer 1 splits fp32 x into two fp16 terms (x = t1 + t2 exactly up to
    ~2^-22 relative) and runs two full-rate fp16 matmuls accumulating into the
    same PSUM tile: ~fp32 accuracy at half fp32-matmul cost.

Distribution: pure data-parallel over 8 NeuronCores (batch 65536 -> 8192/core),
weights replicated, 3 tiny AllReduces for the batch statistics.
"""

import sys
import threading

import numpy as np

TRN_REPO = "/opt/trn_rl_repo"
if TRN_REPO not in sys.path:
    sys.path.insert(0, TRN_REPO)

EPS = 1e-5
N_CORES = 8
B = 65536
BC = B // N_CORES          # 8192 rows per core
D0, D1, D2, D3 = 256, 512, 512, 4
NB = 512                   # batch chunk (one PSUM bank of fp32)
NCH = BC // NB             # 16 chunks per core
K1 = D0 // 128             # 2 contraction tiles, layer 1
F1 = D1 // 128             # 4 output tiles, layer 1
K2 = D1 // 128             # 4
F2 = D2 // 128             # 4
K3 = D2 // 128             # 4
GRP = 16                   # phase-T groups (512 rows each)

_LOCK = threading.Lock()
_CACHE = {}


def _build(alpha1, alpha2, alpha3, n_cores=N_CORES, phase=99, dbg=False):
    import concourse.bacc as bacc
    import concourse.mybir as mybir
    import concourse.tile as tile
    import concourse.masks as masks

    dt = mybir.dt
    AF = mybir.ActivationFunctionType
    OP = mybir.AluOpType

    nc = bacc.Bacc("TRN2", target_bir_lowering=False, debug=False,
                   num_devices=n_cores)
    x_in = nc.declare_dram_parameter("x", [BC, D0], dt.float32, isOutput=False)
    w1t_in = nc.declare_dram_parameter("w1t", [D0, D1], dt.float32, isOutput=False)
    w2t_in = nc.declare_dram_parameter("w2t", [D1, D2], dt.float32, isOutput=False)
    w3t_in = nc.declare_dram_parameter("w3t", [D2, D3], dt.float32, isOutput=False)
    out_t = nc.declare_dram_parameter("outT", [D3, BC], dt.float32, isOutput=True)
    if dbg:
        d_sumx = nc.declare_dram_parameter("d_sumx", [128, K1], dt.float32, isOutput=True)
        d_bias1 = nc.declare_dram_parameter("d_bias1", [128, F1], dt.float32, isOutput=True)
        d_xt1 = nc.declare_dram_parameter("d_xt1", [128, NB], dt.float32, isOutput=True)
        d_xt2 = nc.declare_dram_parameter("d_xt2", [128, NB], dt.float32, isOutput=True)
        d_x2 = nc.declare_dram_parameter("d_x2", [128, NB], dt.float32, isOutput=True)
        d_bias2 = nc.declare_dram_parameter("d_bias2", [128, F2], dt.float32, isOutput=True)
        d_y3 = nc.declare_dram_parameter("d_y3", [D3, BC], dt.float32, isOutput=True)
        d_gst3 = nc.declare_dram_parameter("d_gst3", [D3, 2], dt.float32, isOutput=True)

    RG = [list(range(n_cores))]
    inv_b = 1.0 / float(B)

    with tile.TileContext(nc, pool_alloc_mode="queue") as tc:
        with (
            tc.tile_pool(name="w", bufs=1) as pw,
            tc.tile_pool(name="dram", bufs=1, space="DRAM") as pd,
        ):
            ident = pw.tile([128, 128], dt.float32, tag="ident")
            masks.make_identity(nc, ident[:])

            # ---------------- weights: load + sign ----------------
            s1t = [pw.tile([128, D1], dt.float16, tag=f"s1t{k}", name=f"s1t{k}") for k in range(K1)]
            s1tf = [pw.tile([128, D1], dt.float32, tag=f"s1tf{k}", name=f"s1tf{k}") for k in range(K1)]
            s2t = [pw.tile([128, D2], dt.bfloat16, tag=f"s2t{k}", name=f"s2t{k}") for k in range(K2)]
            s2tf = [pw.tile([128, D2], dt.float32, tag=f"s2tf{k}", name=f"s2tf{k}") for k in range(K2)]
            s3t = [pw.tile([128, D3], dt.bfloat16, tag=f"s3t{k}", name=f"s3t{k}") for k in range(K3)]

            cm_wst = tc.tile_pool(name="wst", bufs=3)
            p_wst = cm_wst.__enter__()
            for k in range(K1):
                wst = p_wst.tile([128, D1], dt.float32, tag="wst")
                nc.sync.dma_start(wst[:], w1t_in[k * 128:(k + 1) * 128, :])
                nc.scalar.sign(s1t[k][:], wst[:])
                nc.scalar.sign(s1tf[k][:], wst[:])
            for k in range(K2):
                wst = p_wst.tile([128, D2], dt.float32, tag="wst")
                nc.sync.dma_start(wst[:], w2t_in[k * 128:(k + 1) * 128, :])
                nc.scalar.sign(s2t[k][:], wst[:])
                nc.scalar.sign(s2tf[k][:], wst[:])
            for k in range(K3):
                wst3 = p_wst.tile([128, D3], dt.float32, tag="wst3")
                nc.sync.dma_start(wst3[:], w3t_in[k * 128:(k + 1) * 128, :])
                nc.scalar.sign(s3t[k][:], wst3[:])
            cm_wst.__exit__(None, None, None)

            # ---------------- phase T: load x, transpose, fp16 split, sums ---
            xt1 = [pw.tile([128, BC], dt.float16, tag=f"xt1_{k}", name=f"xt1_{k}") for k in range(K1)]
            xt2 = [pw.tile([128, BC], dt.float16, tag=f"xt2_{k}", name=f"xt2_{k}") for k in range(K1)]
            xsA = pw.tile([128, K1 * GRP], dt.float32, tag="xsA")

            cm_stage = tc.tile_pool(name="stage", bufs=3)
            p_st = cm_stage.__enter__()
            cm_pstr = tc.tile_pool(name="pstr", bufs=3, space="PSUM")
            p_pstr = cm_pstr.__enter__()

            x_r = x_in.rearrange("(g t p) f -> g p t f", p=128, t=4)
            for g in range(GRP):
                xa = p_st.tile([128, 4 * D0], dt.float32, tag="xa")
                xa3 = xa.rearrange("p (t f) -> p t f", t=4)
                nc.sync.dma_start(xa3[:], x_r[g])
                for k in range(K1):
                    ps = p_pstr.tile([128, 512], dt.float32, tag="pstr")
                    for t in range(4):
                        nc.tensor.transpose(
                            ps[:, t * 128:(t + 1) * 128],
                            xa3[:, t, k * 128:(k + 1) * 128],
                            ident[:])
                    gs = slice(g * 512, (g + 1) * 512)
                    col = k * GRP + g
                    nc.scalar.activation(xt1[k][:, gs], ps[:], AF.Copy)
                    nc.vector.tensor_tensor(
                        xt2[k][:, gs], ps[:], xt1[k][:, gs], OP.subtract)
                    nc.vector.tensor_reduce(
                        xsA[:, col:col + 1], ps[:],
                        axis=mybir.AxisListType.X, op=OP.add)
            cm_stage.__exit__(None, None, None)
            cm_pstr.__exit__(None, None, None)

            # ---------------- AllReduce #1: sum(x) ----------------
            sumx = pw.tile([128, K1], dt.float32, tag="sumx")
            nc.vector.tensor_reduce(
                sumx[:], xsA.rearrange("p (k g) -> p k g", k=K1),
                axis=mybir.AxisListType.X, op=OP.add)
            ar1i = pd.tile([128, K1], dt.float32, tag="ar1i")
            ar1o = pd.tile([128, K1], dt.float32, tag="ar1o")
            nc.sync.dma_start(ar1i[:], sumx[:])
            nc.gpsimd.collective_compute(
                "AllReduce", OP.add, replica_groups=RG,
                ins=[ar1i.opt()], outs=[ar1o.opt()])
            gsumx = pw.tile([128, K1], dt.float32, tag="gsumx")
            nc.sync.dma_start(gsumx[:], ar1o[:])

            # mu1 matvec: bias1[:, fo] = -(sum(x) @ s1)[fo] / B
            bias1 = pw.tile([128, F1], dt.float32, tag="bias1")
            cm_pmu1 = tc.tile_pool(name="pmu1", bufs=1, space="PSUM")
            p_pmu1 = cm_pmu1.__enter__()
            for fo in range(F1):
                pm = p_pmu1.tile([128, 1], dt.float32, tag="pm1")
                for k in range(K1):
                    nc.tensor.matmul(
                        pm[:], s1tf[k][:, fo * 128:(fo + 1) * 128],
                        gsumx[:, k:k + 1],
                        start=(k == 0), stop=(k == K1 - 1))
                nc.scalar.mul(bias1[:, fo:fo + 1], pm[:], -inv_b)
            cm_pmu1.__exit__(None, None, None)

            # ---------------- layer 1 ----------------
            x2 = [pw.tile([128, BC], dt.bfloat16, tag=f"x2_{f}", name=f"x2_{f}") for f in range(F1)]
            x2s = pw.tile([128, F1 * NCH], dt.float32, tag="x2s")

            cm_ps1 = tc.tile_pool(name="ps1", bufs=6, space="PSUM")
            p_ps1 = cm_ps1.__enter__()
            for c in range(NCH):
                cs = slice(c * NB, (c + 1) * NB)
                for fo in range(F1):
                    fos = slice(fo * 128, (fo + 1) * 128)
                    ps1 = p_ps1.tile([128, NB], dt.float32, tag="ps1")
                    terms = [(s1t[k], xt1[k]) for k in range(K1)] + \
                            [(s1t[k], xt2[k]) for k in range(K1)]
                    for i, (w, xv) in enumerate(terms):
                        nc.tensor.matmul(
                            ps1[:], w[:, fos], xv[:, cs],
                            start=(i == 0), stop=(i == len(terms) - 1))
                    nc.scalar.activation(
                        x2[fo][:, cs], ps1[:], AF.Sign,
                        bias=bias1[:, fo:fo + 1],
                        accum_out=x2s[:, fo * NCH + c:fo * NCH + c + 1])
            cm_ps1.__exit__(None, None, None)

            # ---------------- AllReduce #2: sum(x2) ----------------
            x2sum = pw.tile([128, F1], dt.float32, tag="x2sum")
            nc.vector.tensor_reduce(
                x2sum[:], x2s.rearrange("p (f c) -> p f c", f=F1),
                axis=mybir.AxisListType.X, op=OP.add)
            ar2i = pd.tile([128, F1], dt.float32, tag="ar2i")
            ar2o = pd.tile([128, F1], dt.float32, tag="ar2o")
            nc.sync.dma_start(ar2i[:], x2sum[:])
            nc.gpsimd.collective_compute(
                "AllReduce", OP.add, replica_groups=RG,
                ins=[ar2i.opt()], outs=[ar2o.opt()])
            gx2sum = pw.tile([128, K2], dt.float32, tag="gx2sum")
            nc.sync.dma_start(gx2sum[:], ar2o[:])

            # ---------------- layers 2+3, chunk-pipelined ----------------
            cm_x3 = tc.tile_pool(name="x3", bufs=2)
            p_x3 = cm_x3.__enter__()
            y3 = pw.tile([D3, BC], dt.float16, tag="y3")
            y3s = pw.tile([D3, NCH], dt.float32, tag="y3s")
            y3q = pw.tile([D3, NCH], dt.float32, tag="y3q")
            sq_scr = pw.tile([D3, NB], dt.float32, tag="sqscr")
            bias2 = pw.tile([128, F2], dt.float32, tag="bias2")

            cm_ps23 = tc.tile_pool(name="ps23", bufs=4, space="PSUM")
            p_ps2 = cm_ps23.__enter__()
            p_ps3 = p_ps2

            for c in range(NCH):
                cs = slice(c * NB, (c + 1) * NB)
                ps2l = []
                for go in range(F2):
                    gos = slice(go * 128, (go + 1) * 128)
                    ps2 = p_ps2.tile([128, NB], dt.float32, tag="ps2")
                    for k in range(K2):
                        nc.tensor.matmul(
                            ps2[:], s2t[k][:, gos], x2[k][:, cs],
                            start=(k == 0), stop=(k == K2 - 1))
                    ps2l.append(ps2)
                if c == 0:
                    # mu2 matvec traced here so PE reaches it after chunk-0
                    # matmuls; it waits on AllReduce #2.
                    for go in range(F2):
                        pm = p_ps2.tile([128, 1], dt.float32, tag="pm2", bufs=1)
                        for k in range(K2):
                            nc.tensor.matmul(
                                pm[:], s2tf[k][:, go * 128:(go + 1) * 128],
                                gx2sum[:, k:k + 1],
                                start=(k == 0), stop=(k == K2 - 1))
                        nc.scalar.mul(bias2[:, go:go + 1], pm[:], -inv_b)
                x3c = []
                for go in range(F2):
                    x3t = p_x3.tile([128, NB], dt.bfloat16, tag=f"x3_{go}", name=f"x3_{go}")
                    nc.scalar.activation(
                        x3t[:], ps2l[go][:], AF.Sign,
                        bias=bias2[:, go:go + 1])
                    x3c.append(x3t)
                ps3 = p_ps3.tile([D3, NB], dt.float32, tag="ps3", bufs=2)
                for k in range(K3):
                    nc.tensor.matmul(
                        ps3[:], s3t[k][:, 0:D3], x3c[k][:],
                        start=(k == 0), stop=(k == K3 - 1))
                nc.scalar.activation(y3[:, cs], ps3[:], AF.Copy)
                nc.vector.tensor_reduce(
                    y3s[:, c:c + 1], ps3[:],
                    axis=mybir.AxisListType.X, op=OP.add)
                nc.vector.tensor_tensor(
                    sq_scr[:], ps3[:], y3[:, cs], OP.mult)
                nc.vector.tensor_reduce(
                    y3q[:, c:c + 1], sq_scr[:],
                    axis=mybir.AxisListType.X, op=OP.add)
            cm_ps23.__exit__(None, None, None)
            cm_x3.__exit__(None, None, None)

            # ---------------- AllReduce #3: sum(y3), sum(y3^2) -----------
            st3 = pw.tile([D3, 2], dt.float32, tag="st3")
            nc.vector.tensor_reduce(
                st3[:, 0:1], y3s[:], axis=mybir.AxisListType.X, op=OP.add)
            nc.vector.tensor_reduce(
                st3[:, 1:2], y3q[:], axis=mybir.AxisListType.X, op=OP.add)
            ar3i = pd.tile([D3, 2], dt.float32, tag="ar3i")
            ar3o = pd.tile([D3, 2], dt.float32, tag="ar3o")
            nc.sync.dma_start(ar3i[:], st3[:])
            nc.gpsimd.collective_compute(
                "AllReduce", OP.add, replica_groups=RG,
                ins=[ar3i.opt()], outs=[ar3o.opt()])
            gst3 = pw.tile([D3, 2], dt.float32, tag="gst3")
            nc.sync.dma_start(gst3[:], ar3o[:])

            # bn3 + prelu: out = prelu((y3 - mu)/sqrt(var+eps), alpha)
            mu3 = pw.tile([D3, 1], dt.float32, tag="mu3")
            ex2 = pw.tile([D3, 1], dt.float32, tag="ex2")
            mu3sq = pw.tile([D3, 1], dt.float32, tag="mu3sq")
            var3 = pw.tile([D3, 1], dt.float32, tag="var3")
            epsT = pw.tile([D3, 1], dt.float32, tag="epsT")
            vare = pw.tile([D3, 1], dt.float32, tag="vare")
            rec = pw.tile([D3, 1], dt.float32, tag="rec")
            scale3 = pw.tile([D3, 1], dt.float32, tag="scale3")
            msc = pw.tile([D3, 1], dt.float32, tag="msc")
            bias3 = pw.tile([D3, 1], dt.float32, tag="bias3")
            nc.scalar.mul(mu3[:], gst3[:, 0:1], inv_b)
            nc.scalar.mul(ex2[:], gst3[:, 1:2], inv_b)
            nc.vector.tensor_tensor(mu3sq[:], mu3[:], mu3[:], OP.mult)
            nc.vector.tensor_tensor(var3[:], ex2[:], mu3sq[:], OP.subtract)
            nc.vector.memset(epsT[:], EPS)
            nc.vector.tensor_tensor(vare[:], var3[:], epsT[:], OP.add)
            nc.vector.reciprocal(rec[:], vare[:])
            nc.scalar.sqrt(scale3[:], rec[:])
            nc.vector.tensor_tensor(msc[:], mu3[:], scale3[:], OP.mult)
            nc.scalar.mul(bias3[:], msc[:], -1.0)

            if dbg:
                dbg_pool = tc.tile_pool(name="dbgp", bufs=1)
                p_dbg = dbg_pool.__enter__()
                nc.sync.dma_start(d_sumx[:], sumx[:])
                nc.sync.dma_start(d_bias1[:], bias1[:])
                for (dst, srcT) in [(d_xt1, xt1[0]), (d_xt2, xt2[0]), (d_x2, x2[0])]:
                    tmpd = p_dbg.tile([128, NB], dt.float32, tag="tmpd", name="tmpd")
                    nc.vector.tensor_copy(tmpd[:], srcT[:, 0:NB])
                    nc.sync.dma_start(dst[:], tmpd[:])
                nc.sync.dma_start(d_bias2[:], bias2[:])
                nc.gpsimd.dma_start(d_y3[:], y3[:])
                nc.sync.dma_start(d_gst3[:], gst3[:])
                dbg_pool.__exit__(None, None, None)
            cm_out = tc.tile_pool(name="out", bufs=1)
            p_out = cm_out.__enter__()
            outsb = p_out.tile([D3, BC], dt.float32, tag="outsb")
            nc.scalar.activation(
                outsb[:], y3[:], AF.Prelu,
                bias=bias3[:, 0:1], scale=scale3[:, 0:1], alpha=float(alpha3))
            nc.sync.dma_start(out_t[:], outsb[:])
            cm_out.__exit__(None, None, None)

    nc.compile()
    return nc


def _make_executable(nc):
    """Build a cached jitted shard_map executable for repeated runs
    (mirrors concourse.bass2jax.run_bass_via_pjrt)."""
    import jax
    import concourse.mybir as mybir
    from concourse import bass2jax
    from jax.experimental.shard_map import shard_map
    from jax.sharding import Mesh, PartitionSpec

    bass2jax.install_neuronx_cc_hook()

    partition_name = (nc.partition_id_tensor.name
                      if nc.partition_id_tensor else None)
    in_names, out_names, out_avals, zero_outs = [], [], [], []
    for alloc in nc.m.functions[0].allocations:
        if not isinstance(alloc, mybir.MemoryLocationSet):
            continue
        if not alloc.memorylocations:
            continue
        name = alloc.memorylocations[0].name
        if alloc.kind == "ExternalInput":
            if name != partition_name:
                in_names.append(name)
        elif alloc.kind == "ExternalOutput":
            shape = tuple(alloc.tensor_shape)
            dtype = mybir.dt.np(alloc.dtype)
            out_names.append(name)
            out_avals.append(jax.core.ShapedArray(shape, dtype))
            zero_outs.append(np.zeros(shape, dtype))
    n_params = len(in_names)
    n_outs = len(out_avals)
    all_in_names = list(in_names) + list(out_names)
    if partition_name is not None:
        all_in_names.append(partition_name)
    donate = tuple(range(n_params, n_params + n_outs))

    def _body(*args):
        operands = list(args)
        if partition_name is not None:
            operands.append(bass2jax.partition_id_tensor())
        outs = bass2jax._bass_exec_p.bind(
            *operands,
            out_avals=tuple(out_avals),
            in_names=tuple(all_in_names),
            out_names=tuple(out_names),
            lowering_input_output_aliases=(),
            sim_require_finite=True,
            sim_require_nnan=True,
            nc=nc,
        )
        return tuple(outs)

    devices = jax.devices()[:N_CORES]
    assert len(devices) == N_CORES, f"need {N_CORES} devices, have {len(jax.devices())}"
    mesh = Mesh(np.asarray(devices), ("core",))
    in_specs = (PartitionSpec("core"),) * (n_params + n_outs)
    out_specs = (PartitionSpec("core"),) * n_outs
    sharded = jax.jit(
        shard_map(_body, mesh=mesh, in_specs=in_specs, out_specs=out_specs,
                  check_rep=False),
        donate_argnums=donate, keep_unused=True)
    return sharded, in_names, out_names, out_avals, zero_outs


def _get_exec(alpha1, alpha2, alpha3):
    key = (float(alpha1), float(alpha2), float(alpha3))
    with _LOCK:
        if key not in _CACHE:
            nc = _build(*key)
            _CACHE[key] = _make_executable(nc)
    return _CACHE[key]


def prepare_inputs(x, W1, W2, W3):
    """Host-side sharding / relayout (no arithmetic): batch-shard x,
    transpose weights, replicate them per core."""
    x = np.ascontiguousarray(np.asarray(x, dtype=np.float32))
    w1t = np.ascontiguousarray(np.asarray(W1, dtype=np.float32).T)
    w2t = np.ascontiguousarray(np.asarray(W2, dtype=np.float32).T)
    w3t = np.ascontiguousarray(np.asarray(W3, dtype=np.float32).T)
    per_core = {
        "x": [x[c * BC:(c + 1) * BC] for c in range(N_CORES)],
        "w1t": [w1t] * N_CORES,
        "w2t": [w2t] * N_CORES,
        "w3t": [w3t] * N_CORES,
    }
    return per_core


def run_sharded(per_core, exec_pack):
    sharded, in_names, out_names, out_avals, zero_outs = exec_pack
    concat_in = [np.concatenate(per_core[name], axis=0) for name in in_names]
    concat_zero = [np.zeros((N_CORES * z.shape[0],) + z.shape[1:], z.dtype)
                   for z in zero_outs]
    out_arrs = sharded(*concat_in, *concat_zero)
    outs = {}
    for i, name in enumerate(out_names):
        full = np.asarray(out_arrs[i]).reshape(
            (N_CORES,) + tuple(out_avals[i].shape))
        outs[name] = full
    return outs


def kernel(x, W1, W2, W3, a1, a2, a3):
    exec_pack = _get_exec(float(a1), float(a2), float(a3))
    per_core = prepare_inputs(x, W1, W2, W3)
    outs = run_sharded(per_core, exec_pack)
    out_t = outs["outT"]                     # [N_CORES, 4, BC]
    out = np.empty((B, D3), dtype=np.float32)
    for c in range(N_CORES):
        out[c * BC:(c + 1) * BC] = out_t[c].T
    return out


# revision 15
# speedup vs baseline: 13917.9359x; 13917.9359x over previous
"""Trainium2 Bass kernel for a 3-layer binarized MLP (BNN) with BatchNorm.

Math (reference):
  layer(x, W, a):  y = x_bin @ sign(W).T ; bn = (y - mean)/sqrt(var + eps) over
  the GLOBAL batch; p = prelu(bn, a); out = sign(p) (except last layer).

Key identities used:
  * sign(prelu((y - mu)/std)) == sign(y - mu)   (std > 0, a > 0) -> layers 1,2
    need only the global per-feature mean, not the variance.
  * mean(y) = mean(x_in) @ sign(W).T -> the cross-core all-reduce of the input
    sums can be computed before/while the layer's matmuls run.
  * layers 2,3 inputs are exactly {-1,+1} so bf16 matmuls are exact.
  * layer 1 splits fp32 x into two fp16 terms (x = t1 + t2 exactly up to
    ~2^-22 relative) and runs two full-rate fp16 matmuls accumulating into the
    same PSUM tile: ~fp32 accuracy at half fp32-matmul cost.

Distribution: pure data-parallel over 8 NeuronCores (batch 65536 -> 8192/core),
weights replicated, 3 tiny AllReduces for the batch statistics.
"""

import sys
import threading

import numpy as np

TRN_REPO = "/opt/trn_rl_repo"
if TRN_REPO not in sys.path:
    sys.path.insert(0, TRN_REPO)

EPS = 1e-5
N_CORES = 8
B = 65536
BC = B // N_CORES          # 8192 rows per core
D0, D1, D2, D3 = 256, 512, 512, 4
NB = 512                   # batch chunk (one PSUM bank of fp32)
NCH = BC // NB             # 16 chunks per core
K1 = D0 // 128             # 2 contraction tiles, layer 1
F1 = D1 // 128             # 4 output tiles, layer 1
K2 = D1 // 128             # 4
F2 = D2 // 128             # 4
K3 = D2 // 128             # 4
GRP = 16                   # phase-T groups (512 rows each)

_LOCK = threading.Lock()
_CACHE = {}


def _build(alpha1, alpha2, alpha3, n_cores=N_CORES, phase=99, dbg=False, reps=1,
           stage1_n=3, stage2_n=4):
    import concourse.bacc as bacc
    import concourse.mybir as mybir
    import concourse.tile as tile
    import concourse.masks as masks

    dt = mybir.dt
    AF = mybir.ActivationFunctionType
    OP = mybir.AluOpType

    nc = bacc.Bacc("TRN2", target_bir_lowering=False, debug=False,
                   num_devices=n_cores)
    x_in = nc.declare_dram_parameter("x", [BC, D0], dt.float32, isOutput=False)
    w1t_in = nc.declare_dram_parameter("w1t", [D0, D1], dt.float32, isOutput=False)
    w2t_in = nc.declare_dram_parameter("w2t", [D1, D2], dt.float32, isOutput=False)
    w3t_in = nc.declare_dram_parameter("w3t", [D2, D3], dt.float32, isOutput=False)
    out_t = nc.declare_dram_parameter("outT", [D3, BC], dt.float32, isOutput=True)
    if dbg:
        d_sumx = nc.declare_dram_parameter("d_sumx", [128, K1], dt.float32, isOutput=True)
        d_bias1 = nc.declare_dram_parameter("d_bias1", [128, F1], dt.float32, isOutput=True)
        d_bias2 = nc.declare_dram_parameter("d_bias2", [128, F2], dt.float32, isOutput=True)
        d_y3 = nc.declare_dram_parameter("d_y3", [D3, BC], dt.float32, isOutput=True)
        d_gst3 = nc.declare_dram_parameter("d_gst3", [D3, 2], dt.float32, isOutput=True)

    RG = [list(range(n_cores))]
    inv_b = 1.0 / float(B)

    with tile.TileContext(nc, pool_alloc_mode="queue") as tc:
        with (
            tc.tile_pool(name="w", bufs=1) as pw,
            tc.tile_pool(name="dram", bufs=1, space="DRAM") as pd,
        ):
            for _rep in range(reps):
                ident = pw.tile([128, 128], dt.float32, tag="ident", name="ident")
                masks.make_identity(nc, ident[:])

                # ---------------- weights: load + sign ----------------
                s1t = [pw.tile([128, D1], dt.float16, tag=f"s1t{k}", name=f"s1t{k}") for k in range(K1)]
                s1tf = [pw.tile([128, D1], dt.float32, tag=f"s1tf{k}", name=f"s1tf{k}") for k in range(K1)]
                s2t = [pw.tile([128, D2], dt.bfloat16, tag=f"s2t{k}", name=f"s2t{k}") for k in range(K2)]
                s2tf = [pw.tile([128, D2], dt.float32, tag=f"s2tf{k}", name=f"s2tf{k}") for k in range(K2)]
                s3t = [pw.tile([128, D3], dt.bfloat16, tag=f"s3t{k}", name=f"s3t{k}") for k in range(K3)]

                cm_wst = tc.tile_pool(name="wst", bufs=3)
                p_wst = cm_wst.__enter__()
                for k in range(K1):
                    wst = p_wst.tile([128, D1], dt.float32, tag="wst", name="wst")
                    nc.sync.dma_start(wst[:], w1t_in[k * 128:(k + 1) * 128, :])
                    nc.scalar.sign(s1t[k][:], wst[:])
                    nc.scalar.sign(s1tf[k][:], wst[:])
                for k in range(K2):
                    wst = p_wst.tile([128, D2], dt.float32, tag="wst", name="wst")
                    nc.sync.dma_start(wst[:], w2t_in[k * 128:(k + 1) * 128, :])
                    nc.scalar.sign(s2t[k][:], wst[:])
                    nc.scalar.sign(s2tf[k][:], wst[:])
                for k in range(K3):
                    wst3 = p_wst.tile([128, D3], dt.float32, tag="wst3", name="wst3")
                    nc.sync.dma_start(wst3[:], w3t_in[k * 128:(k + 1) * 128, :])
                    nc.scalar.sign(s3t[k][:], wst3[:])
                cm_wst.__exit__(None, None, None)

                # ---- persistent activation tiles
                xt1 = [pw.tile([128, BC], dt.float16, tag=f"xt1_{k}", name=f"xt1_{k}") for k in range(K1)]
                xt2 = [pw.tile([128, BC], dt.float16, tag=f"xt2_{k}", name=f"xt2_{k}") for k in range(K1)]
                xsA = pw.tile([128, K1 * GRP], dt.float32, tag="xsA", name="xsA")
                x2 = [pw.tile([128, BC], dt.bfloat16, tag=f"x2_{f}", name=f"x2_{f}") for f in range(F1)]
                x2s = pw.tile([128, F1 * NCH], dt.float32, tag="x2s", name="x2s")
                bias1 = pw.tile([128, F1], dt.float32, tag="bias1", name="bias1")
                bias2 = pw.tile([128, F2], dt.float32, tag="bias2", name="bias2")
                y3 = pw.tile([D3, BC], dt.float16, tag="y3", name="y3")
                y3s = pw.tile([D3, NCH], dt.float32, tag="y3s", name="y3s")
                y3q = pw.tile([D3, NCH], dt.float32, tag="y3q", name="y3q")
                sq_scr = pw.tile([D3, NB], dt.float32, tag="sqscr", name="sqscr")

                # ============ phase T + early L1, interleaved per group ======
                cm_y1st = tc.tile_pool(name="y1st", bufs=1)
                p_y1st = cm_y1st.__enter__()
                y1st = [[p_y1st.tile([128, NB], dt.float32,
                                     tag=f"y1st_{g}_{fo}", name=f"y1st_{g}_{fo}")
                         for fo in range(F1)] for g in range(stage1_n)]

                cm_stage = tc.tile_pool(name="stage", bufs=2)
                p_st = cm_stage.__enter__()
                cm_psA = tc.tile_pool(name="psA", bufs=2, space="PSUM")
                p_psA = cm_psA.__enter__()

                x_r = x_in.rearrange("(g t p) f -> g p t f", p=128, t=4)

                def phase_t_group(g):
                    xa = p_st.tile([128, 4 * D0], dt.float32, tag="xa", name="xa")
                    xa3 = xa.rearrange("p (t f) -> p t f", t=4)
                    nc.sync.dma_start(xa3[:], x_r[g])
                    for k in range(K1):
                        ps = p_psA.tile([128, 512], dt.float32, tag="pstr", bufs=2,
                                        name="pstr")
                        for t in range(4):
                            nc.tensor.transpose(
                                ps[:, t * 128:(t + 1) * 128],
                                xa3[:, t, k * 128:(k + 1) * 128],
                                ident[:])
                        gs = slice(g * 512, (g + 1) * 512)
                        col = k * GRP + g
                        nc.scalar.activation(
                            xt1[k][:, gs], ps[:], AF.Copy,
                            accum_out=xsA[:, col:col + 1])
                        nc.vector.tensor_tensor(
                            xt2[k][:, gs], ps[:], xt1[k][:, gs], OP.subtract)

                def l1_mms(c, ps1):
                    cs = slice(c * NB, (c + 1) * NB)
                    for fo in range(F1):
                        fsl = slice(fo * 128, (fo + 1) * 128)
                        pt = ps1[fo]
                        terms = []
                        for k in range(K1):
                            terms.append((s1t[k][:, fsl], xt1[k][:, cs]))
                            terms.append((s1t[k][:, fsl], xt2[k][:, cs]))
                        for i, (w, xv) in enumerate(terms):
                            nc.tensor.matmul(
                                pt[:], w, xv,
                                start=(i == 0), stop=(i == len(terms) - 1))

                def l1_sign(c, src_tiles):
                    cs = slice(c * NB, (c + 1) * NB)
                    for fo in range(F1):
                        nc.scalar.activation(
                            x2[fo][:, cs], src_tiles[fo][:], AF.Sign,
                            bias=bias1[:, fo:fo + 1],
                            accum_out=x2s[:, fo * NCH + c:fo * NCH + c + 1])

                # groups 0..GRP-1; for g < stage1_n also run L1 mms + stage y1
                for g in range(GRP):
                    phase_t_group(g)
                    if g < stage1_n:
                        ps1 = [p_psA.tile([128, NB], dt.float32, tag="ps1", bufs=5,
                                          name="ps1") for _ in range(F1)]
                        l1_mms(g, ps1)
                        for fo in range(F1):
                            nc.vector.tensor_copy(y1st[g][fo][:], ps1[fo][:])
                cm_stage.__exit__(None, None, None)

                # ---------------- AllReduce #1: sum(x) ----------------
                sumx = pw.tile([128, K1], dt.float32, tag="sumx", name="sumx")
                nc.vector.tensor_reduce(
                    sumx[:], xsA.rearrange("p (k g) -> p k g", k=K1),
                    axis=mybir.AxisListType.X, op=OP.add)
                ar1i = pd.tile([128, K1], dt.float32, tag="ar1i", name="ar1i")
                ar1o = pd.tile([128, K1], dt.float32, tag="ar1o", name="ar1o")
                nc.sync.dma_start(ar1i[:], sumx[:])
                nc.gpsimd.collective_compute(
                    "AllReduce", OP.add, replica_groups=RG,
                    ins=[ar1i.opt()], outs=[ar1o.opt()])
                gsumx = pw.tile([128, K1], dt.float32, tag="gsumx", name="gsumx")
                nc.sync.dma_start(gsumx[:], ar1o[:])

                # run-ahead L1 chunks while AllReduce #1 is in flight
                ahead = []
                for c in range(stage1_n, min(stage1_n + 1, NCH)):
                    ps1 = [p_psA.tile([128, NB], dt.float32, tag="ps1", bufs=5,
                                      name="ps1") for _ in range(F1)]
                    l1_mms(c, ps1)
                    ahead.append((c, ps1))

                # mu1 matvec: bias1[:, fo] = -(sum(x) @ s1)[fo] / B
                for fo in range(F1):
                    pm = p_psA.tile([128, 1], dt.float32, tag="pm1", bufs=1,
                                    name="pm1")
                    for k in range(K1):
                        nc.tensor.matmul(
                            pm[:], s1tf[k][:, fo * 128:(fo + 1) * 128],
                            gsumx[:, k:k + 1],
                            start=(k == 0), stop=(k == K1 - 1))
                    nc.scalar.mul(bias1[:, fo:fo + 1], pm[:], -inv_b)

                # drain staged + run-ahead chunks, then the rest
                for g in range(stage1_n):
                    l1_sign(g, y1st[g])
                for (c, ps1) in ahead:
                    l1_sign(c, ps1)
                for c in range(stage1_n + len(ahead), NCH):
                    ps1 = [p_psA.tile([128, NB], dt.float32, tag="ps1", bufs=5,
                                      name="ps1") for _ in range(F1)]
                    l1_mms(c, ps1)
                    l1_sign(c, ps1)
                cm_psA.__exit__(None, None, None)
                cm_y1st.__exit__(None, None, None)

                # ---------------- AllReduce #2: sum(x2) ----------------
                x2sum = pw.tile([128, F1], dt.float32, tag="x2sum", name="x2sum")
                nc.vector.tensor_reduce(
                    x2sum[:], x2s.rearrange("p (f c) -> p f c", f=F1),
                    axis=mybir.AxisListType.X, op=OP.add)
                ar2i = pd.tile([128, F1], dt.float32, tag="ar2i", name="ar2i")
                ar2o = pd.tile([128, F1], dt.float32, tag="ar2o", name="ar2o")
                nc.sync.dma_start(ar2i[:], x2sum[:])
                nc.gpsimd.collective_compute(
                    "AllReduce", OP.add, replica_groups=RG,
                    ins=[ar2i.opt()], outs=[ar2o.opt()])
                gx2sum = pw.tile([128, K2], dt.float32, tag="gx2sum", name="gx2sum")
                nc.sync.dma_start(gx2sum[:], ar2o[:])

                # ---------------- layers 2+3 ----------------
                cm_y2st = tc.tile_pool(name="y2st", bufs=1)
                p_y2st = cm_y2st.__enter__()
                y2st = [[p_y2st.tile([128, NB], dt.float16,
                                     tag=f"y2st_{g}_{go}", name=f"y2st_{g}_{go}")
                         for go in range(F2)] for g in range(stage2_n)]
                cm_x3 = tc.tile_pool(name="x3", bufs=2)
                p_x3 = cm_x3.__enter__()
                cm_psB = tc.tile_pool(name="psB", bufs=4, space="PSUM")
                p_psB = cm_psB.__enter__()

                def l2_mms(c):
                    cs = slice(c * NB, (c + 1) * NB)
                    out = []
                    for go in range(F2):
                        gos = slice(go * 128, (go + 1) * 128)
                        ps2 = p_psB.tile([128, NB], dt.float32, tag="ps2", bufs=4,
                                         name="ps2")
                        for k in range(K2):
                            nc.tensor.matmul(
                                ps2[:], s2t[k][:, gos], x2[k][:, cs],
                                start=(k == 0), stop=(k == K2 - 1))
                        out.append(ps2)
                    return out

                def l23_tail(c, src_tiles):
                    # sign -> x3, L3 matmul, y3 copy + stats
                    cs = slice(c * NB, (c + 1) * NB)
                    x3c = []
                    for go in range(F2):
                        x3t = p_x3.tile([128, NB], dt.bfloat16, tag=f"x3_{go}",
                                        name=f"x3_{go}")
                        nc.scalar.activation(
                            x3t[:], src_tiles[go][:], AF.Sign,
                            bias=bias2[:, go:go + 1])
                        x3c.append(x3t)
                    ps3 = p_psB.tile([D3, NB], dt.float32, tag="ps3", bufs=2,
                                     name="ps3")
                    for k in range(K3):
                        nc.tensor.matmul(
                            ps3[:], s3t[k][:, 0:D3], x3c[k][:],
                            start=(k == 0), stop=(k == K3 - 1))
                    nc.scalar.activation(y3[:, cs], ps3[:], AF.Copy)
                    nc.vector.tensor_reduce(
                        y3s[:, c:c + 1], ps3[:],
                        axis=mybir.AxisListType.X, op=OP.add)
                    nc.vector.tensor_tensor(
                        sq_scr[:], ps3[:], y3[:, cs], OP.mult)
                    nc.vector.tensor_reduce(
                        y3q[:, c:c + 1], sq_scr[:],
                        axis=mybir.AxisListType.X, op=OP.add)

                # staged chunks: mms + copy to y2st (fp16 exact for ints)
                for g in range(stage2_n):
                    ps2l = l2_mms(g)
                    for go in range(F2):
                        nc.vector.tensor_copy(y2st[g][go][:], ps2l[go][:])

                # mu2 matvec (waits on AllReduce #2)
                for go in range(F2):
                    pm = p_psB.tile([128, 1], dt.float32, tag="pm2", bufs=1,
                                    name="pm2")
                    for k in range(K2):
                        nc.tensor.matmul(
                            pm[:], s2tf[k][:, go * 128:(go + 1) * 128],
                            gx2sum[:, k:k + 1],
                            start=(k == 0), stop=(k == K2 - 1))
                    nc.scalar.mul(bias2[:, go:go + 1], pm[:], -inv_b)

                for g in range(stage2_n):
                    l23_tail(g, y2st[g])
                for c in range(stage2_n, NCH):
                    ps2l = l2_mms(c)
                    l23_tail(c, ps2l)
                cm_psB.__exit__(None, None, None)
                cm_x3.__exit__(None, None, None)
                cm_y2st.__exit__(None, None, None)

                # ---------------- AllReduce #3 + bn3 + prelu -------------
                st3 = pw.tile([D3, 2], dt.float32, tag="st3", name="st3")
                nc.vector.tensor_reduce(
                    st3[:, 0:1], y3s[:], axis=mybir.AxisListType.X, op=OP.add)
                nc.vector.tensor_reduce(
                    st3[:, 1:2], y3q[:], axis=mybir.AxisListType.X, op=OP.add)
                ar3i = pd.tile([D3, 2], dt.float32, tag="ar3i", name="ar3i")
                ar3o = pd.tile([D3, 2], dt.float32, tag="ar3o", name="ar3o")
                nc.sync.dma_start(ar3i[:], st3[:])
                nc.gpsimd.collective_compute(
                    "AllReduce", OP.add, replica_groups=RG,
                    ins=[ar3i.opt()], outs=[ar3o.opt()])
                gst3 = pw.tile([D3, 2], dt.float32, tag="gst3", name="gst3")
                nc.sync.dma_start(gst3[:], ar3o[:])

                mu3 = pw.tile([D3, 1], dt.float32, tag="mu3", name="mu3")
                ex2 = pw.tile([D3, 1], dt.float32, tag="ex2", name="ex2")
                mu3sq = pw.tile([D3, 1], dt.float32, tag="mu3sq", name="mu3sq")
                var3 = pw.tile([D3, 1], dt.float32, tag="var3", name="var3")
                epsT = pw.tile([D3, 1], dt.float32, tag="epsT", name="epsT")
                vare = pw.tile([D3, 1], dt.float32, tag="vare", name="vare")
                rec = pw.tile([D3, 1], dt.float32, tag="rec", name="rec")
                scale3 = pw.tile([D3, 1], dt.float32, tag="scale3", name="scale3")
                msc = pw.tile([D3, 1], dt.float32, tag="msc", name="msc")
                bias3 = pw.tile([D3, 1], dt.float32, tag="bias3", name="bias3")
                nc.scalar.mul(mu3[:], gst3[:, 0:1], inv_b)
                nc.scalar.mul(ex2[:], gst3[:, 1:2], inv_b)
                nc.vector.tensor_tensor(mu3sq[:], mu3[:], mu3[:], OP.mult)
                nc.vector.tensor_tensor(var3[:], ex2[:], mu3sq[:], OP.subtract)
                nc.vector.memset(epsT[:], EPS)
                nc.vector.tensor_tensor(vare[:], var3[:], epsT[:], OP.add)
                nc.vector.reciprocal(rec[:], vare[:])
                nc.scalar.sqrt(scale3[:], rec[:])
                nc.vector.tensor_tensor(msc[:], mu3[:], scale3[:], OP.mult)
                nc.scalar.mul(bias3[:], msc[:], -1.0)

                if dbg:
                    nc.sync.dma_start(d_sumx[:], sumx[:])
                    nc.sync.dma_start(d_bias1[:], bias1[:])
                    nc.sync.dma_start(d_bias2[:], bias2[:])
                    nc.gpsimd.dma_start(d_y3[:], y3[:])
                    nc.sync.dma_start(d_gst3[:], gst3[:])

                cm_out = tc.tile_pool(name="out", bufs=1)
                p_out = cm_out.__enter__()
                # split the final prelu + store into 4 pieces so ACT and DMA
                # overlap
                QB = BC // 4
                for q in range(4):
                    qs = slice(q * QB, (q + 1) * QB)
                    outsb = p_out.tile([D3, QB], dt.float32, tag="outsb",
                                       bufs=2, name="outsb")
                    nc.scalar.activation(
                        outsb[:], y3[:, qs], AF.Prelu,
                        bias=bias3[:, 0:1], scale=scale3[:, 0:1],
                        alpha=float(alpha3))
                    nc.sync.dma_start(out_t[:, qs], outsb[:])
                cm_out.__exit__(None, None, None)

    nc.compile()
    return nc


def _make_executable(nc):
    """Build a cached jitted shard_map executable for repeated runs
    (mirrors concourse.bass2jax.run_bass_via_pjrt)."""
    import jax
    import concourse.mybir as mybir
    from concourse import bass2jax
    from jax.experimental.shard_map import shard_map
    from jax.sharding import Mesh, PartitionSpec

    bass2jax.install_neuronx_cc_hook()

    partition_name = (nc.partition_id_tensor.name
                      if nc.partition_id_tensor else None)
    in_names, out_names, out_avals, zero_outs = [], [], [], []
    for alloc in nc.m.functions[0].allocations:
        if not isinstance(alloc, mybir.MemoryLocationSet):
            continue
        if not alloc.memorylocations:
            continue
        name = alloc.memorylocations[0].name
        if alloc.kind == "ExternalInput":
            if name != partition_name:
                in_names.append(name)
        elif alloc.kind == "ExternalOutput":
            shape = tuple(alloc.tensor_shape)
            dtype = mybir.dt.np(alloc.dtype)
            out_names.append(name)
            out_avals.append(jax.core.ShapedArray(shape, dtype))
            zero_outs.append(np.zeros(shape, dtype))
    n_params = len(in_names)
    n_outs = len(out_avals)
    all_in_names = list(in_names) + list(out_names)
    if partition_name is not None:
        all_in_names.append(partition_name)
    donate = tuple(range(n_params, n_params + n_outs))

    def _body(*args):
        operands = list(args)
        if partition_name is not None:
            operands.append(bass2jax.partition_id_tensor())
        outs = bass2jax._bass_exec_p.bind(
            *operands,
            out_avals=tuple(out_avals),
            in_names=tuple(all_in_names),
            out_names=tuple(out_names),
            lowering_input_output_aliases=(),
            sim_require_finite=True,
            sim_require_nnan=True,
            nc=nc,
        )
        return tuple(outs)

    devices = jax.devices()[:N_CORES]
    assert len(devices) == N_CORES, f"need {N_CORES} devices, have {len(jax.devices())}"
    mesh = Mesh(np.asarray(devices), ("core",))
    in_specs = (PartitionSpec("core"),) * (n_params + n_outs)
    out_specs = (PartitionSpec("core"),) * n_outs
    sharded = jax.jit(
        shard_map(_body, mesh=mesh, in_specs=in_specs, out_specs=out_specs,
                  check_rep=False),
        donate_argnums=donate, keep_unused=True)
    return sharded, in_names, out_names, out_avals, zero_outs


def _get_exec(alpha1, alpha2, alpha3):
    key = (float(alpha1), float(alpha2), float(alpha3))
    with _LOCK:
        if key not in _CACHE:
            nc = _build(*key)
            _CACHE[key] = _make_executable(nc)
    return _CACHE[key]


def prepare_inputs(x, W1, W2, W3):
    """Host-side sharding / relayout (no arithmetic): batch-shard x,
    transpose weights, replicate them per core."""
    x = np.ascontiguousarray(np.asarray(x, dtype=np.float32))
    w1t = np.ascontiguousarray(np.asarray(W1, dtype=np.float32).T)
    w2t = np.ascontiguousarray(np.asarray(W2, dtype=np.float32).T)
    w3t = np.ascontiguousarray(np.asarray(W3, dtype=np.float32).T)
    per_core = {
        "x": [x[c * BC:(c + 1) * BC] for c in range(N_CORES)],
        "w1t": [w1t] * N_CORES,
        "w2t": [w2t] * N_CORES,
        "w3t": [w3t] * N_CORES,
    }
    return per_core


def run_sharded(per_core, exec_pack):
    sharded, in_names, out_names, out_avals, zero_outs = exec_pack
    concat_in = [np.concatenate(per_core[name], axis=0) for name in in_names]
    concat_zero = [np.zeros((N_CORES * z.shape[0],) + z.shape[1:], z.dtype)
                   for z in zero_outs]
    out_arrs = sharded(*concat_in, *concat_zero)
    outs = {}
    for i, name in enumerate(out_names):
        full = np.asarray(out_arrs[i]).reshape(
            (N_CORES,) + tuple(out_avals[i].shape))
        outs[name] = full
    return outs


def kernel(x, W1, W2, W3, a1, a2, a3):
    exec_pack = _get_exec(float(a1), float(a2), float(a3))
    per_core = prepare_inputs(x, W1, W2, W3)
    outs = run_sharded(per_core, exec_pack)
    out_t = outs["outT"]                     # [N_CORES, 4, BC]
    out = np.empty((B, D3), dtype=np.float32)
    for c in range(N_CORES):
        out[c * BC:(c + 1) * BC] = out_t[c].T
    return out
